# revision 1
# baseline (speedup 1.0000x reference)
"""BuildCostVolume Trainium2 kernel.

Reference computation (per batch b):
  uh = shear_d(attn_map_uh)            # shift along d by (32-h), zero-fill
  vw = shear_d(attn_map_vw, per w)     # shift along d by (32-w), zero-fill
  out[b,n,a,k,h,w] = sum_d P[a,k,d] * sheared[n][b,a,d,h,w]   # P: [9,21,128] pool matrix

Key transform: substituting j = d - (32 - t)  (t = h for uh, w for vw) turns
shear+pool into a per-t matmul with a *shifted* pool matrix:
  out[k,t,:] = sum_j Ppad[a][k, j + 64 - t] * x[j, t, :]
where Ppad zero-pads P's d-axis by 32 on both sides (handles both shear
validity masking and pool-window clipping exactly).

So: shard b across 8 cores.  Host transposes vw to [b,a,d,w,h] so the shear
axis is free-major for both halves.  Each (n,a) tile is 64 small matmuls
(K=128 -> M=21, N=64) using per-shift weight slices; weight tensors are
built by a sliding-window (overlapping) DMA from a tiny [192,21] source.
Outputs of 4 tiles are packed at PSUM partition offsets {0,32,64,96}
(legal col-group bases) so PSUM->SBUF copies and the store DMA use wide
partition counts.
"""

import numpy as np

import concourse.bass as bass
import concourse.bacc as bacc
import concourse.mybir as mybir
import concourse.tile as tile
from concourse.bass_utils import run_bass_kernel_spmd

F32 = mybir.dt.float32
F16 = mybir.dt.float16

DISP_RANGE = 10
OUT_D = 2 * DISP_RANGE + 1  # 21
B, A, D, H, W = 8, 9, 128, 64, 64
HW = H * W  # 4096
NCORES = 8
NBLK = 2 * A  # 18 blocks per core: bi = n*9 + a
# a -> index into the 4 distinct pool matrices (delta = max(|a-4|,1))
DELTA_IDX = [3, 2, 1, 0, 0, 0, 1, 2, 3]
WSRC_ROWS = 192  # 128 + 2*32 zero padding on the d axis
WCOLS = 64 * OUT_D  # 1344 free elements per weight tile

TRACE = False  # set by test.py for profiling runs
LAST_RESULTS = None  # BassKernelResults of the most recent run

_COMPILED = None


def _pool_matrix():
    # [9, 21, 128]; same as reference._pool_matrix(9, 128)
    P = np.zeros((A, OUT_D, D), dtype=np.float32)
    for i in range(A):
        a_delta = max(abs(i - A // 2), 1)
        L = 2 * DISP_RANGE * a_delta + 1
        start0 = D // 2 - DISP_RANGE * a_delta
        for k in range(OUT_D):
            s = (k * L) // OUT_D
            e = -((-(k + 1) * L) // OUT_D)
            P[i, k, start0 + s : start0 + e] = 1.0 / (e - s)
    return P


def _build_wsrc():
    # [4, 192, 21]: for each distinct delta, PpadT[r, k] = Ppad[k, r]
    # where Ppad[:, 32:160] = P[a_repr], zeros elsewhere.
    P = _pool_matrix()
    wsrc = np.zeros((4, WSRC_ROWS, OUT_D), dtype=np.float32)
    for di, a_repr in enumerate([3, 2, 1, 0]):  # deltas 1,2,3,4
        pad = np.zeros((OUT_D, WSRC_ROWS), dtype=np.float32)
        pad[:, 32 : 32 + D] = P[a_repr]
        wsrc[di] = pad.T
    return wsrc


def _build_nc():
    nc = bacc.Bacc("TRN2", target_bir_lowering=False)

    x_uh = nc.declare_dram_parameter("x_uh", [A, D, HW], F16, isOutput=False)
    x_vw = nc.declare_dram_parameter("x_vw", [A, D, HW], F16, isOutput=False)
    wsrc = nc.declare_dram_parameter("wsrc", [4, WSRC_ROWS, OUT_D], F16, isOutput=False)
    out = nc.declare_dram_parameter("out", [NBLK * OUT_D, HW], F32, isOutput=True)

    wsrc_h = wsrc.tensor if isinstance(wsrc, bass.AP) else wsrc

    blocks = [(n, a) for n in (0, 1) for a in range(A)]
    quads = [blocks[i : i + 4] for i in range(0, NBLK, 4)]

    with tile.TileContext(nc) as tc:
        with (
            tc.tile_pool(name="wpool", bufs=1) as wp,
            tc.tile_pool(name="xpool", bufs=12) as xp,
            tc.tile_pool(name="opool", bufs=2) as op,
            tc.tile_pool(name="psum", bufs=8, space="PSUM") as pp,
        ):
            # Load the 4 weight tiles once: partition j holds the 64*21
            # window Ppad.T[j+1 : j+65, :] (sliding-window replication).
            wts = []
            for di in range(4):
                wt = wp.tile([D, WCOLS], F16, tag=f"w{di}", name=f"wt{di}")
                src = bass.AP(
                    wsrc_h,
                    di * (WSRC_ROWS * OUT_D) + OUT_D,
                    [[OUT_D, D], [1, WCOLS]],
                )
                nc.scalar.dma_start(out=wt[:], in_=src)
                wts.append(wt)

            for qi, quad in enumerate(quads):
                pts = [pp.tile([D, 512], F32, tag="ps", name=f"pt{qi}_{g}") for g in range(8)]

                for ti, (n, a) in enumerate(quad):
                    xt = xp.tile([D, HW], F16, tag="x", name=f"xt{qi}_{n}_{a}")
                    src = (x_uh if n == 0 else x_vw)[a]
                    nc.sync.dma_start(out=xt[:], in_=src)
                    wt = wts[DELTA_IDX[a]]
                    p0 = 32 * ti
                    for h in range(64):
                        g, r = divmod(h, 8)
                        hr = 63 - h  # weight windows stored h-reversed
                        nc.tensor.matmul(
                            out=pts[g][p0 : p0 + OUT_D, 64 * r : 64 * r + 64],
                            lhsT=wt[:, OUT_D * hr : OUT_D * hr + OUT_D],
                            rhs=xt[:, 64 * h : 64 * h + 64],
                            start=True,
                            stop=True,
                            tile_position=(0, p0),
                        )

                osb = op.tile([128, HW], F32, tag="o", name=f"osb{qi}")
                for g in range(8):
                    nc.vector.tensor_copy(
                        out=osb[:, 512 * g : 512 * g + 512], in_=pts[g][:]
                    )

                for ti in range(len(quad)):
                    bi = 4 * qi + ti
                    nc.scalar.dma_start(
                        out=out[OUT_D * bi : OUT_D * bi + OUT_D],
                        in_=osb[32 * ti : 32 * ti + OUT_D, :],
                    )

    nc.compile()
    return nc


def _get_compiled():
    global _COMPILED
    if _COMPILED is None:
        _COMPILED = _build_nc()
    return _COMPILED


def kernel(attn_map_uh, attn_map_vw):
    global LAST_RESULTS
    attn_map_uh = np.ascontiguousarray(np.asarray(attn_map_uh, dtype=np.float16))
    vwt = np.ascontiguousarray(
        np.swapaxes(np.asarray(attn_map_vw, dtype=np.float16), -1, -2)
    )
    wsrc = _build_wsrc().astype(np.float16)

    nc = _get_compiled()
    in_maps = [
        {
            "x_uh": attn_map_uh[c].reshape(A, D, HW),
            "x_vw": vwt[c].reshape(A, D, HW),
            "wsrc": wsrc,
        }
        for c in range(NCORES)
    ]
    res = run_bass_kernel_spmd(nc, in_maps, list(range(NCORES)), trace=TRACE)
    LAST_RESULTS = res

    out = np.empty((B, 2, A, OUT_D, H, W), dtype=np.float32)
    for c in range(NCORES):
        o = res.results[c]["out"].reshape(2, A, OUT_D, H, W)
        out[c, 0] = o[0]
        out[c, 1] = np.swapaxes(o[1], -1, -2)
    return out

